# revision 1
# baseline (speedup 1.0000x reference)
"""Quantized Linear (8-bit act / 4-bit weight fake-quant) on 8 Trainium2 cores.

Math (per reference):
  xq = rne(x / s_x) * s_x          s_x = max(absmax(x)/127, 1e-8)
  wq = rne(w / s_w) * s_w          s_w = max(absmax(w)/7,   1e-8)
  bq = rne(b / s_b) * s_b          s_b = max(absmax(b)/127, 1e-8)
  out_pre = bq + xq @ wq.T
  out = rne(out_pre / s_o) * s_o   s_o = max(absmax(out_pre)/127, 1e-8)

Device strategy (2 token-groups x 4 outf-groups over 8 cores): see the
build() body; this is the empirically fastest variant measured (v6):
partition-major host tiling, exclusive-slice absmax through big pool
slots, split w/x AllReduce-max collectives with prefetch of the w-jc0 /
x-g0 first halves during the collective window, ACT/DVE alternating
magic-constant quantization, j-tile-outer matmuls into 8 PSUM banks,
fp16 out_pre buffer, fused-scale eviction on ACT.
"""

import sys

sys.path.insert(0, "/opt/trn_rl_repo")

import numpy as np

import concourse.bass as bass
import concourse.mybir as mybir
import concourse.tile as tile
from concourse import bacc, bass_isa

F32 = mybir.dt.float32
F16 = mybir.dt.float16
BF16 = mybir.dt.bfloat16
AF = mybir.ActivationFunctionType
ALU = mybir.AluOpType
AX = mybir.AxisListType

MAGIC = 12582912.0  # 1.5 * 2**23: fp32 add rounds to nearest-even integer
EPS = 1e-8
INV_QA = float(np.float32(1.0) / np.float32(127.0))
INV_QW = float(np.float32(1.0) / np.float32(7.0))

P = 128


def build(n_cores=8, T=4096, K=4096, J=4096):
    """SPMD program; host rolls each core's columns so that the exclusive
    absmax sub-slices are always the local leading 512 columns."""
    NTG, NJG = 2, 4
    TS = T // NTG            # 2048 tokens per core
    JS = J // NJG            # 1024 out-features per core
    n_kp = K // P            # 32 k-tiles
    GT = 512                 # token group width
    n_g = TS // GT           # 4 token groups
    n_jt = JS // P           # 8 j-tiles
    SB = 2                   # k-tiles per quantize op
    n_sb = n_kp // SB        # 16 sub-blocks per 512-wide column group
    HK = n_kp // 2           # 16: k-tiles in a 32KB prefetch tile

    nc = bacc.Bacc(
        "TRN2", target_bir_lowering=False, debug=False, num_devices=n_cores
    )

    xg_d = nc.dram_tensor("xg", [n_g, P, n_kp, GT], F32, kind="ExternalInput")
    wg_d = nc.dram_tensor("wg", [2, P, n_kp, 512], F32, kind="ExternalInput")
    b_d = nc.dram_tensor("b_full", [J], F32, kind="ExternalInput")
    bs_d = nc.dram_tensor("b_shard", [JS], F32, kind="ExternalInput")
    og_d = nc.dram_tensor("og", [JS // P, P, TS], F32, kind="ExternalOutput")
    ccw_in = nc.dram_tensor("ccw_in", [1, 1], F32)
    ccw_out = nc.dram_tensor("ccw_out", [1, 1], F32)
    ccx_in = nc.dram_tensor("ccx_in", [1, 1], F32)
    ccx_out = nc.dram_tensor("ccx_out", [1, 1], F32)
    cc2_in = nc.dram_tensor("cc2_in", [1, 1], F32)
    cc2_out = nc.dram_tensor("cc2_out", [1, 1], F32)
    groups = [list(range(n_cores))]

    with tile.TileContext(nc) as tc:
        with (
            tc.tile_pool(name="const", bufs=1) as const,
            tc.tile_pool(name="scal", bufs=1) as scal,
            tc.tile_pool(name="wq", bufs=1) as wqp,
            tc.tile_pool(name="xq", bufs=2) as xqp,
            tc.tile_pool(name="op", bufs=1) as opp,
            tc.tile_pool(name="xst", bufs=2) as xst,
            tc.tile_pool(name="wst", bufs=2) as wst,
            tc.tile_pool(name="mid", bufs=3) as midp,
            tc.tile_pool(name="outst", bufs=4) as outst,
            tc.tile_pool(name="mm", bufs=8, space="PSUM") as mmps,
        ):
            magic_t = const.tile([P, 1], F32)
            nc.vector.memset(magic_t[:], MAGIC)
            nmagic_t = const.tile([P, 1], F32)
            nc.vector.memset(nmagic_t[:], -MAGIC)

            # ---------------- Phase A: exclusive-slice absmax ----------------
            # W first: its collective fires while the x slice still streams.
            amx = scal.tile([P, 4], F32)
            amw = scal.tile([P, 4], F32)
            wraw = wqp.tile([P, n_kp, 512], F32, tag="wq", name="wraw")
            for q in range(4):
                nc.scalar.dma_start(
                    wraw[:, q * 8 : (q + 1) * 8, :], wg_d[0, :, q * 8 : (q + 1) * 8, :]
                )
                nc.vector.tensor_reduce(
                    amw[:, q : q + 1], wraw[:, q * 8 : (q + 1) * 8, :],
                    axis=AX.XY, op=ALU.max, apply_absolute_value=True,
                )
            m1w = scal.tile([P, 1], F32)
            nc.vector.tensor_reduce(m1w[:], amw[:], axis=AX.X, op=ALU.max)
            gw = scal.tile([P, 1], F32)
            nc.gpsimd.partition_all_reduce(
                gw[:], m1w[:], channels=P, reduce_op=bass_isa.ReduceOp.max
            )
            nc.gpsimd.dma_start(ccw_in[:], gw[:1, :])
            nc.gpsimd.collective_compute(
                "AllReduce", ALU.max, replica_groups=groups,
                ins=[ccw_in[:]], outs=[ccw_out[:]],
            )

            xr = [
                opp.tile([P, HK, GT], F32, tag="op", name=f"xr{h}") for h in range(2)
            ]
            for h in range(2):
                for q in range(2):
                    nc.scalar.dma_start(
                        xr[h][:, q * 8 : (q + 1) * 8, :],
                        xg_d[0, :, h * HK + q * 8 : h * HK + (q + 1) * 8, :],
                    )
                    nc.vector.tensor_reduce(
                        amx[:, h * 2 + q : h * 2 + q + 1],
                        xr[h][:, q * 8 : (q + 1) * 8, :],
                        axis=AX.XY, op=ALU.max, apply_absolute_value=True,
                    )
            m1x = scal.tile([P, 1], F32)
            nc.vector.tensor_reduce(m1x[:], amx[:], axis=AX.X, op=ALU.max)
            gxp = scal.tile([P, 1], F32)
            nc.gpsimd.partition_all_reduce(
                gxp[:], m1x[:], channels=P, reduce_op=bass_isa.ReduceOp.max
            )
            nc.gpsimd.dma_start(ccx_in[:], gxp[:1, :])
            nc.gpsimd.collective_compute(
                "AllReduce", ALU.max, replica_groups=groups,
                ins=[ccx_in[:]], outs=[ccx_out[:]],
            )

            # Prefetch first halves of w-jc0 and x-g0 during the collectives.
            wpre = opp.tile([P, HK, 512], F32, tag="op", name="wpre")
            nc.scalar.dma_start(wpre[:], wg_d[0, :, 0:HK, :])
            xpre = xqp.tile([P, HK, GT], F32, tag="qx", name="xpre")
            nc.sync.dma_start(xpre[:], xg_d[0, :, 0:HK, :])

            # b absmax (full b available on every core; no collective needed)
            bfull = scal.tile([P, J // P], F32)
            nc.sync.dma_start(bfull[:], b_d.rearrange("(p a) -> p a", p=P))
            bmax0 = scal.tile([P, 1], F32)
            nc.vector.tensor_reduce(
                bmax0[:], bfull[:], axis=AX.X, op=ALU.max,
                apply_absolute_value=True,
            )

            # w scales
            gwx = scal.tile([P, 1], F32)
            nc.gpsimd.dma_start(gwx[:1, :], ccw_out[:])
            bcw = scal.tile([P, 1], F32)
            nc.gpsimd.partition_broadcast(bcw[:], gwx[:1, :], channels=P)
            s_w = scal.tile([P, 1], F32)
            nc.vector.tensor_scalar(s_w[:], bcw[:], INV_QW, EPS, op0=ALU.mult, op1=ALU.max)
            inv_sw = scal.tile([P, 1], F32)
            nc.vector.reciprocal(inv_sw[:], s_w[:])

            # x scales
            gxx = scal.tile([P, 1], F32)
            nc.gpsimd.dma_start(gxx[:1, :], ccx_out[:])
            bcx = scal.tile([P, 1], F32)
            nc.gpsimd.partition_broadcast(bcx[:], gxx[:1, :], channels=P)
            s_x = scal.tile([P, 1], F32)
            nc.vector.tensor_scalar(s_x[:], bcx[:], INV_QA, EPS, op0=ALU.mult, op1=ALU.max)
            inv_sx = scal.tile([P, 1], F32)
            nc.vector.reciprocal(inv_sx[:], s_x[:])
            s_xw = scal.tile([P, 1], F32)
            nc.vector.tensor_tensor(out=s_xw[:], in0=s_x[:], in1=s_w[:], op=ALU.mult)

            # b scales + quantized bias
            bmax = scal.tile([P, 1], F32)
            nc.gpsimd.partition_all_reduce(
                bmax[:], bmax0[:], channels=P, reduce_op=bass_isa.ReduceOp.max
            )
            s_b = scal.tile([P, 1], F32)
            nc.vector.tensor_scalar(s_b[:], bmax[:], INV_QA, EPS, op0=ALU.mult, op1=ALU.max)
            inv_sb = scal.tile([P, 1], F32)
            nc.vector.reciprocal(inv_sb[:], s_b[:])
            bsh = scal.tile([P, n_jt], F32)
            nc.sync.dma_start(bsh[:], bs_d.rearrange("(a p) -> p a", p=P))
            by = scal.tile([P, n_jt], F32)
            nc.scalar.activation(by[:], bsh[:], AF.Identity, bias=magic_t[:], scale=inv_sb[:])
            bq = scal.tile([P, n_jt], F32)
            nc.vector.tensor_scalar(bq[:], by[:], -MAGIC, s_b[:], op0=ALU.add, op1=ALU.mult)

            # ---------------- Quantize helpers (ACT/DVE alternating) ---------
            def quant_chain(src_ap, dst_ap, inv_s, parity):
                mid = midp.tile(list(src_ap.shape), F32, tag="mid")
                if parity == 0:
                    nc.scalar.activation(
                        mid[:], src_ap, AF.Identity, bias=magic_t[:], scale=inv_s[:]
                    )
                    nc.vector.tensor_scalar(dst_ap, mid[:], -MAGIC, None, op0=ALU.add)
                else:
                    nc.vector.tensor_scalar(
                        mid[:], src_ap, inv_s[:], MAGIC, op0=ALU.mult, op1=ALU.add
                    )
                    nc.scalar.activation(
                        dst_ap, mid[:], AF.Identity, bias=nmagic_t[:], scale=1.0
                    )

            # ---------------- W prep / X prep ---------------------------------
            qwT = wqp.tile([P, n_kp, JS], BF16, tag="wq", name="qwT")

            def wprep_jc(jc):
                for sb in range(n_sb):
                    k0 = sb * SB
                    if jc == 0 and k0 < HK:
                        src = wpre[:, k0 : k0 + SB, :]
                    else:
                        wf = wst.tile([P, SB, 512], F32, tag="wst")
                        nc.scalar.dma_start(
                            wf[:], wg_d[jc, :, k0 : k0 + SB, :]
                        )
                        src = wf[:]
                    quant_chain(
                        src,
                        qwT[:, k0 : k0 + SB, jc * 512 : (jc + 1) * 512],
                        inv_sw, sb % 2,
                    )

            def prep_group(g):
                qx = xqp.tile([P, n_kp, GT], BF16, tag="qx", name=f"qx_{g}")
                for sb in range(n_sb):
                    k0 = sb * SB
                    if g == 0 and k0 < HK:
                        src = xpre[:, k0 : k0 + SB, :]
                    else:
                        xf = xst.tile([P, SB, GT], F32, tag="xst")
                        nc.sync.dma_start(
                            xf[:], xg_d[g, :, k0 : k0 + SB, :]
                        )
                        src = xf[:]
                    quant_chain(src, qx[:, k0 : k0 + SB, :], inv_sx, sb % 2)
                return qx

            opre = None
            omax = scal.tile([P, n_g * n_jt], F32)

            def mm_group(g, qx):
                for jt in range(n_jt):
                    bank = mmps.tile([P, GT], F32, tag="mm", name=f"mm_{g}_{jt}")
                    for kt in range(n_kp):
                        nc.tensor.matmul(
                            bank[:],
                            lhsT=qwT[:, kt, jt * P : (jt + 1) * P],
                            rhs=qx[:, kt, :],
                            start=(kt == 0),
                            stop=(kt == n_kp - 1),
                        )
                    oc = opre[:, jt, g * GT : (g + 1) * GT]
                    nc.scalar.activation(
                        oc, bank[:], AF.Identity, bias=bq[:, jt : jt + 1],
                        scale=s_xw[:],
                    )
                    nc.vector.tensor_reduce(
                        omax[:, g * n_jt + jt : g * n_jt + jt + 1], oc,
                        axis=AX.X, op=ALU.max, apply_absolute_value=True,
                    )

            wprep_jc(0)
            qx_tiles = {0: prep_group(0)}
            opre = opp.tile([P, n_jt, TS], F16, tag="op", name="opre")
            wprep_jc(1)
            qx_tiles[1] = prep_group(1)
            for g in range(n_g):
                mm_group(g, qx_tiles.pop(g))
                if g + 2 < n_g:
                    qx_tiles[g + 2] = prep_group(g + 2)

            # ---------------- Tail: global out absmax -> requantize ---------
            om1 = scal.tile([P, 1], F32)
            nc.vector.tensor_reduce(om1[:], omax[:], axis=AX.X, op=ALU.max)
            omr = scal.tile([P, 1], F32)
            nc.gpsimd.partition_all_reduce(
                omr[:], om1[:], channels=P, reduce_op=bass_isa.ReduceOp.max
            )
            nc.sync.dma_start(cc2_in[:], omr[:1, :])
            nc.gpsimd.collective_compute(
                "AllReduce", ALU.max, replica_groups=groups,
                ins=[cc2_in[:]], outs=[cc2_out[:]],
            )
            go = scal.tile([P, 1], F32)
            nc.sync.dma_start(go[:1, :], cc2_out[:])
            bco = scal.tile([P, 1], F32)
            nc.gpsimd.partition_broadcast(bco[:], go[:1, :], channels=P)
            s_o = scal.tile([P, 1], F32)
            nc.vector.tensor_scalar(s_o[:], bco[:], INV_QA, EPS, op0=ALU.mult, op1=ALU.max)
            inv_so = scal.tile([P, 1], F32)
            nc.vector.reciprocal(inv_so[:], s_o[:])

            HW = 1024
            ci = 0
            for jt in range(n_jt):
                for h in range(TS // HW):
                    oy = midp.tile([P, HW], F32, tag="mid")
                    nc.scalar.activation(
                        oy[:], opre[:, jt, h * HW : (h + 1) * HW],
                        AF.Identity, bias=magic_t[:], scale=inv_so[:],
                    )
                    res = outst.tile([P, HW], F32, tag="outst")
                    nc.vector.tensor_scalar(
                        res[:], oy[:], -MAGIC, s_o[:], op0=ALU.add, op1=ALU.mult
                    )
                    eng = nc.scalar if ci % 2 == 0 else nc.sync
                    eng.dma_start(og_d[jt, :, h * HW : (h + 1) * HW], res[:])
                    ci += 1

    nc.compile()
    return nc


def _tile_pmajor(a2d, n_groups, gw):
    """[K, n_groups*gw] -> [n_groups, 128, K//128, gw] partition-major."""
    K = a2d.shape[0]
    return np.ascontiguousarray(
        a2d.reshape(K // 128, 128, n_groups, gw).transpose(2, 1, 0, 3)
    )


def _run(nc, inputs, n_cores, T, K, J, trace=False):
    from concourse.bass_utils import run_bass_kernel_spmd

    NTG, NJG = 2, 4
    TS, JS = T // NTG, J // NJG
    x = np.ascontiguousarray(inputs["x"], dtype=np.float32)
    w = np.ascontiguousarray(inputs["weight"], dtype=np.float32)
    b = np.ascontiguousarray(inputs["b"], dtype=np.float32)
    xT = np.ascontiguousarray(x.T)
    wT = np.ascontiguousarray(w.T)
    in_maps = []
    for c in range(n_cores):
        tg, jgr = divmod(c, NJG)
        xs = xT[:, tg * TS : (tg + 1) * TS]
        ws = wT[:, jgr * JS : (jgr + 1) * JS]
        bs = b[jgr * JS : (jgr + 1) * JS]
        # roll so the exclusive absmax sub-slice is the leading 512 columns
        xrr = np.roll(xs, -jgr * 512, axis=1)
        wrr = np.roll(ws, -tg * 512, axis=1)
        in_maps.append(
            {
                "xg": _tile_pmajor(xrr, TS // 512, 512),
                "wg": _tile_pmajor(wrr, JS // 512, 512),
                "b_full": b,
                "b_shard": np.ascontiguousarray(np.roll(bs, -tg * 512)),
            }
        )
    res = run_bass_kernel_spmd(nc, in_maps, core_ids=list(range(n_cores)), trace=trace)
    out = np.empty((T, J), dtype=np.float32)
    for c in range(n_cores):
        tg, jgr = divmod(c, NJG)
        o = res.results[c]["og"].reshape(JS, TS)  # j rolled by tg*512, t rolled
        o = np.roll(o, tg * 512, axis=0)
        o = np.roll(o, jgr * 512, axis=1)
        out[tg * TS : (tg + 1) * TS, jgr * JS : (jgr + 1) * JS] = o.T
    return out, res


_NC_CACHE = {}


def kernel(**inputs) -> np.ndarray:
    n_cores, T, K, J = 8, 4096, 4096, 4096
    key = (n_cores, T, K, J)
    if key not in _NC_CACHE:
        _NC_CACHE[key] = build(n_cores, T, K, J)
    out, _ = _run(_NC_CACHE[key], inputs, n_cores, T, K, J)
    return out



# revision 2
# speedup vs baseline: 1.0833x; 1.0833x over previous
"""Quantized Linear (8-bit act / 4-bit weight fake-quant) on 8 Trainium2 cores.

Math (per reference):
  xq = rne(x / s_x) * s_x          s_x = max(absmax(x)/127, 1e-8)
  wq = rne(w / s_w) * s_w          s_w = max(absmax(w)/7,   1e-8)
  bq = rne(b / s_b) * s_b          s_b = max(absmax(b)/127, 1e-8)
  out_pre = bq + xq @ wq.T
  out = rne(out_pre / s_o) * s_o   s_o = max(absmax(out_pre)/127, 1e-8)

v7 design (2 token-groups x 4 outf-groups over 8 cores):
 - x cast to fp16 on host (halves x HBM traffic; integer quantization
   absorbs the rounding, measured rel-err stays ~1e-2 < 2e-2).
 - prologue: absmax slice reads get full DMA bandwidth (no competing
   prefetch), chunked DVE reduces pipeline with DMA, ONE combined [w,x]
   AllReduce-max instead of two serial meshes, cc scalars on HWDGE
   rings, dummy warmup collective at t=0 to align cores.
 - kt-outer / jt-inner matmul order: every quantized 2-kt chunk feeds
   8 matmuls immediately -> PE starts ~25us after scales arrive and
   never starves on the quantize chain.
 - PSUM evictions (bank*s_xw + bq -> fp16) on DVE via dual-AP
   tensor_scalar, keeping ACT free for quantize pass work.
 - fp16 output buffer + fp16 DRAM writes (halved write traffic).
"""

import sys

sys.path.insert(0, "/opt/trn_rl_repo")

import numpy as np

import concourse.bass as bass
import concourse.mybir as mybir
import concourse.tile as tile
from concourse import bacc, bass_isa

F32 = mybir.dt.float32
F16 = mybir.dt.float16
BF16 = mybir.dt.bfloat16
AF = mybir.ActivationFunctionType
ALU = mybir.AluOpType
AX = mybir.AxisListType

MAGIC = 12582912.0  # 1.5 * 2**23: fp32 add rounds to nearest-even integer
EPS = 1e-8
INV_QA = float(np.float32(1.0) / np.float32(127.0))
INV_QW = float(np.float32(1.0) / np.float32(7.0))

P = 128


def build(n_cores=8, T=4096, K=4096, J=4096):
    """SPMD program; host rolls each core's columns so that the exclusive
    absmax sub-slices are always the local leading 512 columns."""
    NTG, NJG = 2, 4
    TS = T // NTG            # 2048 tokens per core
    JS = J // NJG            # 1024 out-features per core
    n_kp = K // P            # 32 k-tiles
    GT = 512                 # token group width
    n_g = TS // GT           # 4 token groups
    n_jt = JS // P           # 8 j-tiles
    SB = 2                   # k-tiles per chunk
    n_ch = n_kp // SB        # 16 chunks per 512-wide column group

    nc = bacc.Bacc(
        "TRN2", target_bir_lowering=False, debug=False, num_devices=n_cores
    )

    xg_d = nc.dram_tensor("xg", [n_g, P, n_kp, GT], F16, kind="ExternalInput")
    wg_d = nc.dram_tensor("wg", [2, P, n_kp, 512], F32, kind="ExternalInput")
    b_d = nc.dram_tensor("b_full", [J], F32, kind="ExternalInput")
    bs_d = nc.dram_tensor("b_shard", [JS], F32, kind="ExternalInput")
    og_d = nc.dram_tensor("og", [n_jt, P, TS], F16, kind="ExternalOutput")
    ccd_in = nc.dram_tensor("ccd_in", [1, 1], F32)
    ccd_out = nc.dram_tensor("ccd_out", [1, 1], F32)
    cc1_in = nc.dram_tensor("cc1_in", [1, 2], F32)
    cc1_out = nc.dram_tensor("cc1_out", [1, 2], F32)
    cc2_in = nc.dram_tensor("cc2_in", [1, 1], F32)
    cc2_out = nc.dram_tensor("cc2_out", [1, 1], F32)
    groups = [list(range(n_cores))]

    with tile.TileContext(nc) as tc:
        with (
            tc.tile_pool(name="const", bufs=1) as const,
            tc.tile_pool(name="scal", bufs=1) as scal,
            tc.tile_pool(name="wst", bufs=4) as wst,
            tc.tile_pool(name="xst", bufs=4) as xst,
            tc.tile_pool(name="mid", bufs=3) as midp,
            tc.tile_pool(name="wq", bufs=1) as wqp,
            tc.tile_pool(name="xq", bufs=2) as xqp,
            tc.tile_pool(name="op", bufs=1) as opp,
            tc.tile_pool(name="outst", bufs=2) as outst,
            tc.tile_pool(name="mm", bufs=8, space="PSUM") as mmps,
        ):
            magic_t = const.tile([P, 1], F32)
            nc.vector.memset(magic_t[:], MAGIC)
            nmagic_t = const.tile([P, 1], F32)
            nc.vector.memset(nmagic_t[:], -MAGIC)

            # ---- dummy warmup collective: aligns the 8 cores and warms
            # the CC rings so the real meshes see minimal skew ----
            nc.sync.dma_start(ccd_in[:], magic_t[:1, :])
            nc.gpsimd.collective_compute(
                "AllReduce", ALU.max, replica_groups=groups,
                ins=[ccd_in[:]], outs=[ccd_out[:]],
            )

            # ---- tiny bias loads first (16KB; negligible ring delay) ----
            bfull = scal.tile([P, J // P], F32)
            nc.sync.dma_start(bfull[:], b_d.rearrange("(p a) -> p a", p=P))
            bsh = scal.tile([P, n_jt], F32)
            nc.sync.dma_start(bsh[:], bs_d.rearrange("(a p) -> p a", p=P))

            # ---------------- Phase A: exclusive-slice absmax ----------------
            # w slice (8MB fp32) on the scalar HWDGE ring, x slice (4MB fp16)
            # on the sync ring; chunked DVE reduces pipeline with the DMAs.
            amw = scal.tile([P, n_ch], F32)
            amx = scal.tile([P, n_ch], F32)
            for i in range(n_ch):
                wf = wst.tile([P, SB, 512], F32, tag="wst", name=f"wa{i}")
                nc.scalar.dma_start(wf[:], wg_d[0, :, i * SB : (i + 1) * SB, :])
                nc.vector.tensor_reduce(
                    amw[:, i : i + 1], wf[:], axis=AX.XY, op=ALU.max,
                    apply_absolute_value=True,
                )
                xf = xst.tile([P, SB, GT], F16, tag="xst", name=f"xa{i}")
                nc.sync.dma_start(xf[:], xg_d[0, :, i * SB : (i + 1) * SB, :])
                nc.vector.tensor_reduce(
                    amx[:, i : i + 1], xf[:], axis=AX.XY, op=ALU.max,
                    apply_absolute_value=True,
                )

            m2 = scal.tile([P, 2], F32)
            nc.vector.tensor_reduce(m2[:, 0:1], amw[:], axis=AX.X, op=ALU.max)
            nc.vector.tensor_reduce(m2[:, 1:2], amx[:], axis=AX.X, op=ALU.max)
            g2 = scal.tile([P, 2], F32)
            nc.gpsimd.partition_all_reduce(
                g2[:], m2[:], channels=P, reduce_op=bass_isa.ReduceOp.max
            )
            # combined [wmax, xmax] mesh: one AllReduce round instead of two
            nc.sync.dma_start(cc1_in[:], g2[:1, :])
            nc.gpsimd.collective_compute(
                "AllReduce", ALU.max, replica_groups=groups,
                ins=[cc1_in[:]], outs=[cc1_out[:]],
            )

            # ---- re-read streams (behind the absmax reads on each ring) ----
            wre = []
            for i in range(2 * n_ch):
                jc, ii = divmod(i, n_ch)
                wf = wst.tile([P, SB, 512], F32, tag="wst", name=f"wr{i}")
                nc.scalar.dma_start(wf[:], wg_d[jc, :, ii * SB : (ii + 1) * SB, :])
                wre.append(wf)

            xre = {}
            for g in range(2):
                for i in range(n_ch):
                    xf = xst.tile([P, SB, GT], F16, tag="xst", name=f"xr{g}_{i}")
                    nc.sync.dma_start(xf[:], xg_d[g, :, i * SB : (i + 1) * SB, :])
                    xre[(g, i)] = xf

            # scale readback on the sync ring (HWDGE receipt is fast);
            # g2/g3 streams queued behind it are not needed until >130us.
            gg = scal.tile([P, 2], F32)
            nc.sync.dma_start(gg[:1, :], cc1_out[:])

            for g in range(2, n_g):
                for i in range(n_ch):
                    xf = xst.tile([P, SB, GT], F16, tag="xst", name=f"xr{g}_{i}")
                    nc.sync.dma_start(xf[:], xg_d[g, :, i * SB : (i + 1) * SB, :])
                    xre[(g, i)] = xf

            bc2 = scal.tile([P, 2], F32)
            nc.gpsimd.partition_broadcast(bc2[:], gg[:1, :], channels=P)
            s_w = scal.tile([P, 1], F32)
            nc.vector.tensor_scalar(s_w[:], bc2[:, 0:1], INV_QW, EPS, op0=ALU.mult, op1=ALU.max)
            inv_sw = scal.tile([P, 1], F32)
            nc.vector.reciprocal(inv_sw[:], s_w[:])
            s_x = scal.tile([P, 1], F32)
            nc.vector.tensor_scalar(s_x[:], bc2[:, 1:2], INV_QA, EPS, op0=ALU.mult, op1=ALU.max)
            inv_sx = scal.tile([P, 1], F32)
            nc.vector.reciprocal(inv_sx[:], s_x[:])
            s_xw = scal.tile([P, 1], F32)
            nc.vector.tensor_tensor(out=s_xw[:], in0=s_x[:], in1=s_w[:], op=ALU.mult)

            # ---------------- Quantize helpers (ACT/DVE alternating) ---------
            def quant_chain(src_ap, dst_ap, inv_s, parity):
                mid = midp.tile([P, SB, 512], F32, tag="mid")
                if parity == 0:
                    nc.scalar.activation(
                        mid[:], src_ap, AF.Identity, bias=magic_t[:], scale=inv_s[:]
                    )
                    nc.vector.tensor_scalar(dst_ap, mid[:], -MAGIC, None, op0=ALU.add)
                else:
                    nc.vector.tensor_scalar(
                        mid[:], src_ap, inv_s[:], MAGIC, op0=ALU.mult, op1=ALU.add
                    )
                    nc.scalar.activation(
                        dst_ap, mid[:], AF.Identity, bias=nmagic_t[:], scale=1.0
                    )

            # ---------------- W/X prep + matmul ------------------------------
            qwT = wqp.tile([P, n_kp, JS], BF16, tag="wq", name="qwT")

            def wprep_jc(jc):
                for i in range(n_ch):
                    k0 = i * SB
                    quant_chain(
                        wre[jc * n_ch + i][:],
                        qwT[:, k0 : k0 + SB, jc * 512 : (jc + 1) * 512],
                        inv_sw, i % 2,
                    )

            def prep_group(g):
                qx = xqp.tile([P, n_kp, GT], BF16, tag="qx", name=f"qx_{g}")
                for i in range(n_ch):
                    k0 = i * SB
                    quant_chain(
                        xre[(g, i)][:], qx[:, k0 : k0 + SB, :], inv_sx, (i + 1) % 2
                    )
                return qx

            # jc0 + g0 interleaved so both streams advance together
            qx0 = xqp.tile([P, n_kp, GT], BF16, tag="qx", name="qx_0")
            for i in range(n_ch):
                k0 = i * SB
                quant_chain(
                    wre[i][:], qwT[:, k0 : k0 + SB, 0:512], inv_sw, i % 2
                )
                quant_chain(
                    xre[(0, i)][:], qx0[:, k0 : k0 + SB, :], inv_sx, (i + 1) % 2
                )

            # bias scale + quantized bias (local; needed by first eviction)
            bmax0 = scal.tile([P, 1], F32)
            nc.vector.tensor_reduce(
                bmax0[:], bfull[:], axis=AX.X, op=ALU.max,
                apply_absolute_value=True,
            )
            bmax = scal.tile([P, 1], F32)
            nc.gpsimd.partition_all_reduce(
                bmax[:], bmax0[:], channels=P, reduce_op=bass_isa.ReduceOp.max
            )
            s_b = scal.tile([P, 1], F32)
            nc.vector.tensor_scalar(s_b[:], bmax[:], INV_QA, EPS, op0=ALU.mult, op1=ALU.max)
            inv_sb = scal.tile([P, 1], F32)
            nc.vector.reciprocal(inv_sb[:], s_b[:])
            by = scal.tile([P, n_jt], F32)
            nc.scalar.activation(by[:], bsh[:], AF.Identity, bias=magic_t[:], scale=inv_sb[:])
            bq = scal.tile([P, n_jt], F32)
            nc.vector.tensor_scalar(bq[:], by[:], -MAGIC, s_b[:], op0=ALU.add, op1=ALU.mult)

            # rest of the quantize streams
            wprep_jc(1)
            qx1 = prep_group(1)

            opre = opp.tile([P, n_jt, TS], F16, tag="op", name="opre")
            omax = scal.tile([P, n_g * n_jt], F32)

            def mm_group(g, qx):
                bks = []
                for jt in range(n_jt):
                    bks.append(mmps.tile([P, GT], F32, tag="mm", name=f"mm_{g}_{jt}"))
                for kt in range(n_kp):
                    for jt in range(n_jt):
                        nc.tensor.matmul(
                            bks[jt][:],
                            lhsT=qwT[:, kt, jt * P : (jt + 1) * P],
                            rhs=qx[:, kt, :],
                            start=(kt == 0),
                            stop=(kt == n_kp - 1),
                        )
                return bks

            def evict_group(g, bks):
                for jt in range(n_jt):
                    oc = opre[:, jt, g * GT : (g + 1) * GT]
                    nc.vector.tensor_scalar(
                        oc, bks[jt][:], s_xw[:], bq[:, jt : jt + 1],
                        op0=ALU.mult, op1=ALU.add,
                    )
                    nc.vector.tensor_reduce(
                        omax[:, g * n_jt + jt : g * n_jt + jt + 1], oc,
                        axis=AX.X, op=ALU.max, apply_absolute_value=True,
                    )

            bks0 = mm_group(0, qx0)
            evict_group(0, bks0)
            qx2 = prep_group(2)
            qx3 = prep_group(3)
            bks1 = mm_group(1, qx1)
            evict_group(1, bks1)
            bks2 = mm_group(2, qx2)
            evict_group(2, bks2)
            bks3 = mm_group(3, qx3)
            evict_group(3, bks3)

            # ---------------- Tail: global out absmax -> requantize ---------
            om1 = scal.tile([P, 1], F32)
            nc.vector.tensor_reduce(om1[:], omax[:], axis=AX.X, op=ALU.max)
            omr = scal.tile([P, 1], F32)
            nc.gpsimd.partition_all_reduce(
                omr[:], om1[:], channels=P, reduce_op=bass_isa.ReduceOp.max
            )
            nc.scalar.dma_start(cc2_in[:], omr[:1, :])
            nc.gpsimd.collective_compute(
                "AllReduce", ALU.max, replica_groups=groups,
                ins=[cc2_in[:]], outs=[cc2_out[:]],
            )
            go = scal.tile([P, 1], F32)
            nc.sync.dma_start(go[:1, :], cc2_out[:])
            bco = scal.tile([P, 1], F32)
            nc.gpsimd.partition_broadcast(bco[:], go[:1, :], channels=P)
            s_o = scal.tile([P, 1], F32)
            nc.vector.tensor_scalar(s_o[:], bco[:], INV_QA, EPS, op0=ALU.mult, op1=ALU.max)
            inv_so = scal.tile([P, 1], F32)
            nc.vector.reciprocal(inv_so[:], s_o[:])
            nbt = scal.tile([P, 1], F32)  # -MAGIC * s_o for the ACT-second path
            nc.vector.tensor_scalar(nbt[:], s_o[:], -MAGIC, None, op0=ALU.mult)

            HW = 1024
            ci = 0
            for jt in range(n_jt):
                for h in range(TS // HW):
                    src = opre[:, jt, h * HW : (h + 1) * HW]
                    res = outst.tile([P, HW], F16, tag="outst")
                    if ci % 2 == 0:
                        oy = midp.tile([P, HW], F32, tag="mid")
                        nc.scalar.activation(
                            oy[:], src, AF.Identity, bias=magic_t[:], scale=inv_so[:]
                        )
                        nc.vector.tensor_scalar(
                            res[:], oy[:], -MAGIC, s_o[:], op0=ALU.add, op1=ALU.mult
                        )
                    else:
                        oy = midp.tile([P, HW], F32, tag="mid")
                        nc.vector.tensor_scalar(
                            oy[:], src, inv_so[:], MAGIC, op0=ALU.mult, op1=ALU.add
                        )
                        nc.scalar.activation(
                            res[:], oy[:], AF.Identity, bias=nbt[:], scale=s_o[:]
                        )
                    eng = nc.sync if ci % 2 == 0 else nc.scalar
                    eng.dma_start(og_d[jt, :, h * HW : (h + 1) * HW], res[:])
                    ci += 1

    nc.compile()
    return nc


def _tile_pmajor(a2d, n_groups, gw):
    """[K, n_groups*gw] -> [n_groups, 128, K//128, gw] partition-major."""
    K = a2d.shape[0]
    return np.ascontiguousarray(
        a2d.reshape(K // 128, 128, n_groups, gw).transpose(2, 1, 0, 3)
    )


def _run(nc, inputs, n_cores, T, K, J, trace=False):
    from concourse.bass_utils import run_bass_kernel_spmd

    NTG, NJG = 2, 4
    TS, JS = T // NTG, J // NJG
    x = np.ascontiguousarray(inputs["x"], dtype=np.float32)
    w = np.ascontiguousarray(inputs["weight"], dtype=np.float32)
    b = np.ascontiguousarray(inputs["b"], dtype=np.float32)
    xT = np.ascontiguousarray(x.T.astype(np.float16))
    wT = np.ascontiguousarray(w.T)
    in_maps = []
    for c in range(n_cores):
        tg, jgr = divmod(c, NJG)
        xs = xT[:, tg * TS : (tg + 1) * TS]
        ws = wT[:, jgr * JS : (jgr + 1) * JS]
        bs = b[jgr * JS : (jgr + 1) * JS]
        # roll so the exclusive absmax sub-slice is the leading 512 columns
        xrr = np.roll(xs, -jgr * 512, axis=1)
        wrr = np.roll(ws, -tg * 512, axis=1)
        in_maps.append(
            {
                "xg": _tile_pmajor(xrr, TS // 512, 512),
                "wg": _tile_pmajor(wrr, JS // 512, 512),
                "b_full": b,
                "b_shard": np.ascontiguousarray(np.roll(bs, -tg * 512)),
            }
        )
    res = run_bass_kernel_spmd(nc, in_maps, core_ids=list(range(n_cores)), trace=trace)
    out = np.empty((T, J), dtype=np.float32)
    for c in range(n_cores):
        tg, jgr = divmod(c, NJG)
        o = res.results[c]["og"].reshape(JS, TS).astype(np.float32)
        o = np.roll(o, tg * 512, axis=0)
        o = np.roll(o, jgr * 512, axis=1)
        out[tg * TS : (tg + 1) * TS, jgr * JS : (jgr + 1) * JS] = o.T
    return out, res


_NC_CACHE = {}


def kernel(**inputs) -> np.ndarray:
    n_cores, T, K, J = 8, 4096, 4096, 4096
    key = (n_cores, T, K, J)
    if key not in _NC_CACHE:
        _NC_CACHE[key] = build(n_cores, T, K, J)
    out, _ = _run(_NC_CACHE[key], inputs, n_cores, T, K, J)
    return out


# revision 8
# speedup vs baseline: 1.0938x; 1.0097x over previous
"""Quantized Linear (8-bit act / 4-bit weight fake-quant) on 8 Trainium2 cores.

Math (per reference):
  xq = rne(x / s_x) * s_x          s_x = max(absmax(x)/127, 1e-8)
  wq = rne(w / s_w) * s_w          s_w = max(absmax(w)/7,   1e-8)
  bq = rne(b / s_b) * s_b          s_b = max(absmax(b)/127, 1e-8)
  out_pre = bq + xq @ wq.T
  out = rne(out_pre / s_o) * s_o   s_o = max(absmax(out_pre)/127, 1e-8)

v7 design (2 token-groups x 4 outf-groups over 8 cores):
 - x cast to fp16 on host (halves x HBM traffic; integer quantization
   absorbs the rounding, measured rel-err stays ~1e-2 < 2e-2).
 - prologue: absmax slice reads get full DMA bandwidth (no competing
   prefetch), chunked DVE reduces pipeline with DMA, ONE combined [w,x]
   AllReduce-max instead of two serial meshes, cc scalars on HWDGE
   rings, dummy warmup collective at t=0 to align cores.
 - kt-outer / jt-inner matmul order: every quantized 2-kt chunk feeds
   8 matmuls immediately -> PE starts ~25us after scales arrive and
   never starves on the quantize chain.
 - PSUM evictions (bank*s_xw + bq -> fp16) on DVE via dual-AP
   tensor_scalar, keeping ACT free for quantize pass work.
 - fp16 output buffer + fp16 DRAM writes (halved write traffic).
"""

import sys

sys.path.insert(0, "/opt/trn_rl_repo")

import numpy as np

import concourse.bass as bass
import concourse.mybir as mybir
import concourse.tile as tile
from concourse import bacc, bass_isa

F32 = mybir.dt.float32
F16 = mybir.dt.float16
BF16 = mybir.dt.bfloat16
AF = mybir.ActivationFunctionType
ALU = mybir.AluOpType
AX = mybir.AxisListType

MAGIC = 12582912.0  # 1.5 * 2**23: fp32 add rounds to nearest-even integer
EPS = 1e-8
INV_QA = float(np.float32(1.0) / np.float32(127.0))
INV_QW = float(np.float32(1.0) / np.float32(7.0))

P = 128


def build(n_cores=8, T=4096, K=4096, J=4096):
    """SPMD program; host rolls each core's columns so that the exclusive
    absmax sub-slices are always the local leading 512 columns."""
    NTG, NJG = 2, 4
    TS = T // NTG            # 2048 tokens per core
    JS = J // NJG            # 1024 out-features per core
    n_kp = K // P            # 32 k-tiles
    GT = 512                 # token group width
    n_g = TS // GT           # 4 token groups
    n_jt = JS // P           # 8 j-tiles
    SB = 2                   # k-tiles per chunk
    n_ch = n_kp // SB        # 16 chunks per 512-wide column group

    nc = bacc.Bacc(
        "TRN2", target_bir_lowering=False, debug=False, num_devices=n_cores
    )

    xg_d = nc.dram_tensor("xg", [n_g, P, n_kp, GT], F16, kind="ExternalInput")
    wg_d = nc.dram_tensor("wg", [2, P, n_kp, 512], F32, kind="ExternalInput")
    b_d = nc.dram_tensor("b_full", [J], F32, kind="ExternalInput")
    bs_d = nc.dram_tensor("b_shard", [JS], F32, kind="ExternalInput")
    og_d = nc.dram_tensor("og", [n_jt, P, TS], F16, kind="ExternalOutput")
    cc1_in = nc.dram_tensor("cc1_in", [1, 2], F32)
    cc1_out = nc.dram_tensor("cc1_out", [1, 2], F32)
    cc2_in = nc.dram_tensor("cc2_in", [1, 1], F32)
    cc2_out = nc.dram_tensor("cc2_out", [1, 1], F32)
    groups = [list(range(n_cores))]

    with tile.TileContext(nc) as tc:
        with (
            tc.tile_pool(name="const", bufs=1) as const,
            tc.tile_pool(name="scal", bufs=1) as scal,
            tc.tile_pool(name="wst", bufs=2) as wst,
            tc.tile_pool(name="xst", bufs=2) as xst,
            tc.tile_pool(name="mid", bufs=3) as midp,
            tc.tile_pool(name="wq", bufs=1) as wqp,
            tc.tile_pool(name="xq", bufs=2) as xqp,
            tc.tile_pool(name="op", bufs=1) as opp,
            tc.tile_pool(name="outst", bufs=2) as outst,
            tc.tile_pool(name="mm", bufs=8, space="PSUM") as mmps,
        ):
            magic_t = const.tile([P, 1], F32)
            nc.vector.memset(magic_t[:], MAGIC)
            nmagic_t = const.tile([P, 1], F32)
            nc.vector.memset(nmagic_t[:], -MAGIC)

            # ---- tiny bias loads first (16KB; negligible ring delay) ----
            bfull = scal.tile([P, J // P], F32)
            nc.sync.dma_start(bfull[:], b_d.rearrange("(p a) -> p a", p=P))
            bsh = scal.tile([P, n_jt], F32)
            nc.sync.dma_start(bsh[:], bs_d.rearrange("(a p) -> p a", p=P))

            # ---------------- Phase A: exclusive-slice absmax ----------------
            # Big 2MB/1MB chunks (amortize per-DMA fixed cost), split across
            # both HWDGE rings. The landing tiles sit in the qwT / qx-slot-0
            # ring slots (zero extra SBUF); x-g0 later quantizes directly
            # from xabs, so x-g0 is never re-read.
            wabs = wqp.tile([P, n_kp, 512], F32, tag="wq", name="wabs")
            xabs = xqp.tile([P, n_kp, GT], F16, tag="qx", name="xabs")
            amw = scal.tile([P, 4], F32)
            amx = scal.tile([P, 4], F32)
            for q in range(4):
                weng = nc.scalar if q % 2 == 0 else nc.sync
                xeng = nc.sync if q % 2 == 0 else nc.scalar
                weng.dma_start(
                    wabs[:, q * 8 : (q + 1) * 8, :], wg_d[0, :, q * 8 : (q + 1) * 8, :]
                )
                nc.vector.tensor_reduce(
                    amw[:, q : q + 1], wabs[:, q * 8 : (q + 1) * 8, :],
                    axis=AX.XY, op=ALU.max, apply_absolute_value=True,
                )
                xeng.dma_start(
                    xabs[:, q * 8 : (q + 1) * 8, :], xg_d[0, :, q * 8 : (q + 1) * 8, :]
                )
                nc.vector.tensor_reduce(
                    amx[:, q : q + 1], xabs[:, q * 8 : (q + 1) * 8, :],
                    axis=AX.XY, op=ALU.max, apply_absolute_value=True,
                )

            m2 = scal.tile([P, 2], F32)
            nc.vector.tensor_reduce(m2[:, 0:1], amw[:], axis=AX.X, op=ALU.max)
            nc.vector.tensor_reduce(m2[:, 1:2], amx[:], axis=AX.X, op=ALU.max)
            g2 = scal.tile([P, 2], F32)
            nc.gpsimd.partition_all_reduce(
                g2[:], m2[:], channels=P, reduce_op=bass_isa.ReduceOp.max
            )
            # combined [wmax, xmax] mesh: one AllReduce round instead of two
            nc.sync.dma_start(cc1_in[:], g2[:1, :])
            nc.gpsimd.collective_compute(
                "AllReduce", ALU.max, replica_groups=groups,
                ins=[cc1_in[:]], outs=[cc1_out[:]],
            )

            # ---- re-read / prefetch streams (behind the absmax reads) ----
            # w: 1MB tiles of 4 k-tiles, all on the scalar ring.
            WCH = 4                      # k-tiles per w DMA tile
            n_wch = n_kp // WCH          # 8 tiles per 512-col group
            wre = []
            for i in range(2 * n_wch):
                jc, ii = divmod(i, n_wch)
                wf = wst.tile([P, WCH, 512], F32, tag="wst", name=f"wr{i}")
                nc.scalar.dma_start(
                    wf[:], wg_d[jc, :, ii * WCH : (ii + 1) * WCH, :]
                )
                wre.append(wf)

            # x: g1..g3 in 0.5MB tiles of 4 k-tiles on the sync ring
            # (g0 quantizes straight from xabs).
            xre = {}
            for i in range(n_wch):
                xf = xst.tile([P, WCH, GT], F16, tag="xst", name=f"xr1_{i}")
                nc.sync.dma_start(xf[:], xg_d[1, :, i * WCH : (i + 1) * WCH, :])
                xre[(1, i)] = xf

            # scale readback on the sync ring (HWDGE receipt is fast);
            # g2/g3 streams queued behind it are not needed until >130us.
            gg = scal.tile([P, 2], F32)
            nc.sync.dma_start(gg[:1, :], cc1_out[:])

            for g in range(2, n_g):
                for i in range(n_wch):
                    xf = xst.tile([P, WCH, GT], F16, tag="xst", name=f"xr{g}_{i}")
                    nc.sync.dma_start(xf[:], xg_d[g, :, i * WCH : (i + 1) * WCH, :])
                    xre[(g, i)] = xf

            bc2 = scal.tile([P, 2], F32)
            nc.gpsimd.partition_broadcast(bc2[:], gg[:1, :], channels=P)
            s_w = scal.tile([P, 1], F32)
            nc.vector.tensor_scalar(s_w[:], bc2[:, 0:1], INV_QW, EPS, op0=ALU.mult, op1=ALU.max)
            inv_sw = scal.tile([P, 1], F32)
            nc.vector.reciprocal(inv_sw[:], s_w[:])
            s_x = scal.tile([P, 1], F32)
            nc.vector.tensor_scalar(s_x[:], bc2[:, 1:2], INV_QA, EPS, op0=ALU.mult, op1=ALU.max)
            inv_sx = scal.tile([P, 1], F32)
            nc.vector.reciprocal(inv_sx[:], s_x[:])
            s_xw = scal.tile([P, 1], F32)
            nc.vector.tensor_tensor(out=s_xw[:], in0=s_x[:], in1=s_w[:], op=ALU.mult)

            # ---------------- Quantize helpers (ACT/DVE alternating) ---------
            def quant_chain(src_ap, dst_ap, inv_s, parity):
                mid = midp.tile([P, SB, 512], F32, tag="mid")
                if parity == 0:
                    nc.scalar.activation(
                        mid[:], src_ap, AF.Identity, bias=magic_t[:], scale=inv_s[:]
                    )
                    nc.vector.tensor_scalar(dst_ap, mid[:], -MAGIC, None, op0=ALU.add)
                else:
                    nc.vector.tensor_scalar(
                        mid[:], src_ap, inv_s[:], MAGIC, op0=ALU.mult, op1=ALU.add
                    )
                    nc.scalar.activation(
                        dst_ap, mid[:], AF.Identity, bias=nmagic_t[:], scale=1.0
                    )

            # ---------------- W/X prep + matmul ------------------------------
            qwT = wqp.tile([P, n_kp, JS], BF16, tag="wq", name="qwT")

            def wsrc(i):  # i-th [P,SB,512] w chunk (0..31 across jc0,jc1)
                return wre[i // 2][:, (i % 2) * SB : (i % 2 + 1) * SB, :]

            def xsrc(g, i):  # i-th [P,SB,GT] x chunk of group g
                if g == 0:
                    return xabs[:, i * SB : (i + 1) * SB, :]
                return xre[(g, i // 2)][:, (i % 2) * SB : (i % 2 + 1) * SB, :]

            def wprep_jc(jc):
                for i in range(n_ch):
                    k0 = i * SB
                    quant_chain(
                        wsrc(jc * n_ch + i),
                        qwT[:, k0 : k0 + SB, jc * 512 : (jc + 1) * 512],
                        inv_sw, i % 2,
                    )

            def prep_group(g, qx=None):
                if qx is None:
                    qx = xqp.tile([P, n_kp, GT], BF16, tag="qx", name=f"qx_{g}")
                for i in range(n_ch):
                    k0 = i * SB
                    quant_chain(
                        xsrc(g, i), qx[:, k0 : k0 + SB, :], inv_sx, (i + 1) % 2
                    )
                return qx

            # jc0 + g0 interleaved so both streams advance together
            qx0 = xqp.tile([P, n_kp, GT], BF16, tag="qx", name="qx_0")
            for i in range(n_ch):
                k0 = i * SB
                quant_chain(
                    wsrc(i), qwT[:, k0 : k0 + SB, 0:512], inv_sw, i % 2
                )
                quant_chain(
                    xsrc(0, i), qx0[:, k0 : k0 + SB, :], inv_sx, (i + 1) % 2
                )

            # bias scale + quantized bias (local; needed by first eviction)
            bmax0 = scal.tile([P, 1], F32)
            nc.vector.tensor_reduce(
                bmax0[:], bfull[:], axis=AX.X, op=ALU.max,
                apply_absolute_value=True,
            )
            bmax = scal.tile([P, 1], F32)
            nc.gpsimd.partition_all_reduce(
                bmax[:], bmax0[:], channels=P, reduce_op=bass_isa.ReduceOp.max
            )
            s_b = scal.tile([P, 1], F32)
            nc.vector.tensor_scalar(s_b[:], bmax[:], INV_QA, EPS, op0=ALU.mult, op1=ALU.max)
            inv_sb = scal.tile([P, 1], F32)
            nc.vector.reciprocal(inv_sb[:], s_b[:])
            by = scal.tile([P, n_jt], F32)
            nc.scalar.activation(by[:], bsh[:], AF.Identity, bias=magic_t[:], scale=inv_sb[:])
            bq = scal.tile([P, n_jt], F32)
            nc.vector.tensor_scalar(bq[:], by[:], -MAGIC, s_b[:], op0=ALU.add, op1=ALU.mult)

            # rest of the quantize streams
            wprep_jc(1)
            qx1 = prep_group(1)

            opre = opp.tile([P, n_jt, TS], F16, tag="op", name="opre")
            omax = scal.tile([P, n_g * n_jt], F32)

            def mm_group(g, qx):
                bks = []
                for jt in range(n_jt):
                    bks.append(mmps.tile([P, GT], F32, tag="mm", name=f"mm_{g}_{jt}"))
                for kt in range(n_kp):
                    for jt in range(n_jt):
                        nc.tensor.matmul(
                            bks[jt][:],
                            lhsT=qwT[:, kt, jt * P : (jt + 1) * P],
                            rhs=qx[:, kt, :],
                            start=(kt == 0),
                            stop=(kt == n_kp - 1),
                        )
                return bks

            def evict_group(g, bks):
                for jt in range(n_jt):
                    oc = opre[:, jt, g * GT : (g + 1) * GT]
                    nc.vector.tensor_scalar(
                        oc, bks[jt][:], s_xw[:], bq[:, jt : jt + 1],
                        op0=ALU.mult, op1=ALU.add,
                    )
                    nc.vector.tensor_reduce(
                        omax[:, g * n_jt + jt : g * n_jt + jt + 1], oc,
                        axis=AX.X, op=ALU.max, apply_absolute_value=True,
                    )

            bks0 = mm_group(0, qx0)
            evict_group(0, bks0)
            qx2 = prep_group(2)
            qx3 = prep_group(3)
            bks1 = mm_group(1, qx1)
            evict_group(1, bks1)
            bks2 = mm_group(2, qx2)
            evict_group(2, bks2)
            bks3 = mm_group(3, qx3)
            evict_group(3, bks3)

            # ---------------- Tail: global out absmax -> requantize ---------
            om1 = scal.tile([P, 1], F32)
            nc.vector.tensor_reduce(om1[:], omax[:], axis=AX.X, op=ALU.max)
            omr = scal.tile([P, 1], F32)
            nc.gpsimd.partition_all_reduce(
                omr[:], om1[:], channels=P, reduce_op=bass_isa.ReduceOp.max
            )
            nc.scalar.dma_start(cc2_in[:], omr[:1, :])
            nc.gpsimd.collective_compute(
                "AllReduce", ALU.max, replica_groups=groups,
                ins=[cc2_in[:]], outs=[cc2_out[:]],
            )
            go = scal.tile([P, 1], F32)
            nc.sync.dma_start(go[:1, :], cc2_out[:])
            bco = scal.tile([P, 1], F32)
            nc.gpsimd.partition_broadcast(bco[:], go[:1, :], channels=P)
            s_o = scal.tile([P, 1], F32)
            nc.vector.tensor_scalar(s_o[:], bco[:], INV_QA, EPS, op0=ALU.mult, op1=ALU.max)
            inv_so = scal.tile([P, 1], F32)
            nc.vector.reciprocal(inv_so[:], s_o[:])
            # fp16 magic: out/s_o is in [-127,127], so 1536+v rounds to the
            # integer grid exactly in fp16 (ulp=1 in [1024,2048)).
            M16 = 1536.0
            m16_t = scal.tile([P, 1], F32)
            nc.vector.memset(m16_t[:], M16)
            nbt = scal.tile([P, 1], F32)  # -M16 * s_o for the ACT-second path
            nc.vector.tensor_scalar(nbt[:], s_o[:], -M16, None, op0=ALU.mult)

            for jt in range(n_jt):
                src = opre[:, jt, :]
                res = outst.tile([P, TS], F16, tag="outst")
                if jt % 2 == 0:
                    oy = midp.tile([P, TS], F16, tag="mid")
                    nc.scalar.activation(
                        oy[:], src, AF.Identity, bias=m16_t[:], scale=inv_so[:]
                    )
                    nc.vector.tensor_scalar(
                        res[:], oy[:], -M16, s_o[:], op0=ALU.add, op1=ALU.mult
                    )
                else:
                    oy = midp.tile([P, TS], F16, tag="mid")
                    nc.vector.tensor_scalar(
                        oy[:], src, inv_so[:], M16, op0=ALU.mult, op1=ALU.add
                    )
                    nc.scalar.activation(
                        res[:], oy[:], AF.Identity, bias=nbt[:], scale=s_o[:]
                    )
                eng = nc.sync if jt % 2 == 0 else nc.scalar
                eng.dma_start(og_d[jt, :, :], res[:])

    nc.compile()
    return nc


def _tile_pmajor(a2d, n_groups, gw):
    """[K, n_groups*gw] -> [n_groups, 128, K//128, gw] partition-major."""
    K = a2d.shape[0]
    return np.ascontiguousarray(
        a2d.reshape(K // 128, 128, n_groups, gw).transpose(2, 1, 0, 3)
    )


def _run(nc, inputs, n_cores, T, K, J, trace=False):
    from concourse.bass_utils import run_bass_kernel_spmd

    NTG, NJG = 2, 4
    TS, JS = T // NTG, J // NJG
    x = np.ascontiguousarray(inputs["x"], dtype=np.float32)
    w = np.ascontiguousarray(inputs["weight"], dtype=np.float32)
    b = np.ascontiguousarray(inputs["b"], dtype=np.float32)
    xT = np.ascontiguousarray(x.T.astype(np.float16))
    wT = np.ascontiguousarray(w.T)
    in_maps = []
    for c in range(n_cores):
        tg, jgr = divmod(c, NJG)
        xs = xT[:, tg * TS : (tg + 1) * TS]
        ws = wT[:, jgr * JS : (jgr + 1) * JS]
        bs = b[jgr * JS : (jgr + 1) * JS]
        # roll so the exclusive absmax sub-slice is the leading 512 columns
        xrr = np.roll(xs, -jgr * 512, axis=1)
        wrr = np.roll(ws, -tg * 512, axis=1)
        in_maps.append(
            {
                "xg": _tile_pmajor(xrr, TS // 512, 512),
                "wg": _tile_pmajor(wrr, JS // 512, 512),
                "b_full": b,
                "b_shard": np.ascontiguousarray(np.roll(bs, -tg * 512)),
            }
        )
    res = run_bass_kernel_spmd(nc, in_maps, core_ids=list(range(n_cores)), trace=trace)
    out = np.empty((T, J), dtype=np.float32)
    for c in range(n_cores):
        tg, jgr = divmod(c, NJG)
        o = res.results[c]["og"].reshape(JS, TS).astype(np.float32)
        o = np.roll(o, tg * 512, axis=0)
        o = np.roll(o, jgr * 512, axis=1)
        out[tg * TS : (tg + 1) * TS, jgr * JS : (jgr + 1) * JS] = o.T
    return out, res


_NC_CACHE = {}


def kernel(**inputs) -> np.ndarray:
    n_cores, T, K, J = 8, 4096, 4096, 4096
    key = (n_cores, T, K, J)
    if key not in _NC_CACHE:
        _NC_CACHE[key] = build(n_cores, T, K, J)
    out, _ = _run(_NC_CACHE[key], inputs, n_cores, T, K, J)
    return out


# revision 13
# speedup vs baseline: 1.2074x; 1.1039x over previous
"""Quantized Linear (8-bit act / 4-bit weight fake-quant) on 8 Trainium2 cores.

Math (per reference):
  xq = rne(x / s_x) * s_x          s_x = max(absmax(x)/127, 1e-8)
  wq = rne(w / s_w) * s_w          s_w = max(absmax(w)/7,   1e-8)
  bq = rne(b / s_b) * s_b          s_b = max(absmax(b)/127, 1e-8)
  out_pre = bq + xq @ wq.T
  out = rne(out_pre / s_o) * s_o   s_o = max(absmax(out_pre)/127, 1e-8)

v7 design (2 token-groups x 4 outf-groups over 8 cores):
 - x cast to fp16 on host (halves x HBM traffic; integer quantization
   absorbs the rounding, measured rel-err stays ~1e-2 < 2e-2).
 - prologue: absmax slice reads get full DMA bandwidth (no competing
   prefetch), chunked DVE reduces pipeline with DMA, ONE combined [w,x]
   AllReduce-max instead of two serial meshes, cc scalars on HWDGE
   rings, dummy warmup collective at t=0 to align cores.
 - kt-outer / jt-inner matmul order: every quantized 2-kt chunk feeds
   8 matmuls immediately -> PE starts ~25us after scales arrive and
   never starves on the quantize chain.
 - PSUM evictions (bank*s_xw + bq -> fp16) on DVE via dual-AP
   tensor_scalar, keeping ACT free for quantize pass work.
 - fp16 output buffer + fp16 DRAM writes (halved write traffic).
"""

import sys

sys.path.insert(0, "/opt/trn_rl_repo")

import numpy as np

import concourse.bass as bass
import concourse.mybir as mybir
import concourse.tile as tile
from concourse import bacc, bass_isa

F32 = mybir.dt.float32
F16 = mybir.dt.float16
BF16 = mybir.dt.bfloat16
AF = mybir.ActivationFunctionType
ALU = mybir.AluOpType
AX = mybir.AxisListType

MAGIC = 12582912.0  # 1.5 * 2**23: fp32 add rounds to nearest-even integer
EPS = 1e-8
INV_QA = float(np.float32(1.0) / np.float32(127.0))
INV_QW = float(np.float32(1.0) / np.float32(7.0))

P = 128


def build(n_cores=8, T=4096, K=4096, J=4096):
    """SPMD program; host rolls each core's columns so that the exclusive
    absmax sub-slices are always the local leading 512 columns."""
    NTG, NJG = 2, 4
    TS = T // NTG            # 2048 tokens per core
    JS = J // NJG            # 1024 out-features per core
    n_kp = K // P            # 32 k-tiles
    GT = 512                 # token group width
    n_g = TS // GT           # 4 token groups
    n_jt = JS // P           # 8 j-tiles
    SB = 2                   # k-tiles per chunk
    n_ch = n_kp // SB        # 16 chunks per 512-wide column group

    nc = bacc.Bacc(
        "TRN2", target_bir_lowering=False, debug=False, num_devices=n_cores
    )

    xg_d = nc.dram_tensor("xg", [n_g, P, n_kp, GT], F16, kind="ExternalInput")
    wg_d = nc.dram_tensor("wg", [2, P, n_kp, 512], F32, kind="ExternalInput")
    b_d = nc.dram_tensor("b_full", [J], F32, kind="ExternalInput")
    bs_d = nc.dram_tensor("b_shard", [JS], F32, kind="ExternalInput")
    og_d = nc.dram_tensor("og", [n_jt, P, TS], F16, kind="ExternalOutput")
    ccp_in = nc.dram_tensor("ccp_in", [1, 1], F32)
    ccp_out = nc.dram_tensor("ccp_out", [1, 1], F32)
    cc1_in = nc.dram_tensor("cc1_in", [1, 2], F32)
    cc1_out = nc.dram_tensor("cc1_out", [1, 2], F32)
    cc2_in = nc.dram_tensor("cc2_in", [1, 1], F32)
    cc2_out = nc.dram_tensor("cc2_out", [1, 1], F32)
    groups = [list(range(n_cores))]

    with tile.TileContext(nc) as tc:
        with (
            tc.tile_pool(name="const", bufs=1) as const,
            tc.tile_pool(name="scal", bufs=1) as scal,
            tc.tile_pool(name="wst", bufs=2) as wst,
            tc.tile_pool(name="xst", bufs=2) as xst,
            tc.tile_pool(name="mid", bufs=3) as midp,
            tc.tile_pool(name="wq", bufs=1) as wqp,
            tc.tile_pool(name="xq", bufs=2) as xqp,
            tc.tile_pool(name="op", bufs=1) as opp,
            tc.tile_pool(name="outst", bufs=2) as outst,
            tc.tile_pool(name="mm", bufs=8, space="PSUM") as mmps,
        ):
            magic_t = const.tile([P, 1], F32)
            nc.vector.memset(magic_t[:], MAGIC)
            nmagic_t = const.tile([P, 1], F32)
            nc.vector.memset(nmagic_t[:], -MAGIC)

            # ---- CC warmup: a pairwise mini-mesh completes before the real
            # combined mesh is triggered, so the real one begins ~1us after
            # its trigger instead of ~11us (cold-CC begin latency) ----
            nc.sync.dma_start(ccp_in[:], magic_t[:1, :])
            nc.gpsimd.collective_compute(
                "AllReduce", ALU.max,
                replica_groups=[[2 * i, 2 * i + 1] for i in range(n_cores // 2)],
                ins=[ccp_in[:]], outs=[ccp_out[:]],
            )

            # ---- tiny bias loads first (16KB; negligible ring delay) ----
            bfull = scal.tile([P, J // P], F32)
            nc.sync.dma_start(bfull[:], b_d.rearrange("(p a) -> p a", p=P))
            bsh = scal.tile([P, n_jt], F32)
            nc.sync.dma_start(bsh[:], bs_d.rearrange("(a p) -> p a", p=P))

            # ---------------- Phase A: exclusive-slice absmax ----------------
            # Big 2MB/1MB chunks (amortize per-DMA fixed cost), split across
            # both HWDGE rings. The landing tiles sit in the qwT / qx-slot-0
            # ring slots (zero extra SBUF); x-g0 later quantizes directly
            # from xabs, so x-g0 is never re-read.
            wabs = wqp.tile([P, n_kp, 512], F32, tag="wq", name="wabs")
            xabs = xqp.tile([P, n_kp, GT], F16, tag="qx", name="xabs")
            amw = scal.tile([P, 4], F32)
            amx = scal.tile([P, 4], F32)
            for q in range(4):
                weng = nc.scalar if q % 2 == 0 else nc.sync
                xeng = nc.sync if q % 2 == 0 else nc.scalar
                weng.dma_start(
                    wabs[:, q * 8 : (q + 1) * 8, :], wg_d[0, :, q * 8 : (q + 1) * 8, :]
                )
                nc.vector.tensor_reduce(
                    amw[:, q : q + 1], wabs[:, q * 8 : (q + 1) * 8, :],
                    axis=AX.XY, op=ALU.max, apply_absolute_value=True,
                )
                xeng.dma_start(
                    xabs[:, q * 8 : (q + 1) * 8, :], xg_d[0, :, q * 8 : (q + 1) * 8, :]
                )
                nc.vector.tensor_reduce(
                    amx[:, q : q + 1], xabs[:, q * 8 : (q + 1) * 8, :],
                    axis=AX.XY, op=ALU.max, apply_absolute_value=True,
                )

            m2 = scal.tile([P, 2], F32)
            nc.vector.tensor_reduce(m2[:, 0:1], amw[:], axis=AX.X, op=ALU.max)
            nc.vector.tensor_reduce(m2[:, 1:2], amx[:], axis=AX.X, op=ALU.max)
            g2 = scal.tile([P, 2], F32)
            nc.gpsimd.partition_all_reduce(
                g2[:], m2[:], channels=P, reduce_op=bass_isa.ReduceOp.max
            )
            # combined [wmax, xmax] mesh: one AllReduce round instead of two
            nc.sync.dma_start(cc1_in[:], g2[:1, :])
            nc.gpsimd.collective_compute(
                "AllReduce", ALU.max, replica_groups=groups,
                ins=[cc1_in[:]], outs=[cc1_out[:]],
            )

            # ---- scale readbacks gate the bulk streams on BOTH rings ----
            # All 8 cores race their absmax reads against shared HBM
            # bandwidth; any bulk prefetch issued before the mesh result
            # steals bandwidth from slower peers' absmax reads and inflates
            # the mesh's peer-wait. FIFO-ordering the bulk DMAs behind a
            # mesh-result readback on each ring keeps the race fair.
            gg = scal.tile([P, 2], F32)
            nc.sync.dma_start(gg[:1, :], cc1_out[:])
            gg_s = scal.tile([P, 2], F32)
            nc.scalar.dma_start(gg_s[:1, :], cc1_out[:])

            # w: 1MB tiles of 4 k-tiles, all on the scalar ring.
            WCH = 4                      # k-tiles per w DMA tile
            n_wch = n_kp // WCH          # 8 tiles per 512-col group
            wre = []
            for i in range(2 * n_wch):
                jc, ii = divmod(i, n_wch)
                wf = wst.tile([P, WCH, 512], F32, tag="wst", name=f"wr{i}")
                nc.scalar.dma_start(
                    wf[:], wg_d[jc, :, ii * WCH : (ii + 1) * WCH, :]
                )
                wre.append(wf)

            # x: g1..g3 in 0.5MB tiles of 4 k-tiles on the sync ring
            # (g0 quantizes straight from xabs).
            xre = {}
            for g in range(1, n_g):
                for i in range(n_wch):
                    xf = xst.tile([P, WCH, GT], F16, tag="xst", name=f"xr{g}_{i}")
                    nc.sync.dma_start(xf[:], xg_d[g, :, i * WCH : (i + 1) * WCH, :])
                    xre[(g, i)] = xf

            bc2 = scal.tile([P, 2], F32)
            nc.gpsimd.partition_broadcast(bc2[:], gg[:1, :], channels=P)
            s_w = scal.tile([P, 1], F32)
            nc.vector.tensor_scalar(s_w[:], bc2[:, 0:1], INV_QW, EPS, op0=ALU.mult, op1=ALU.max)
            inv_sw = scal.tile([P, 1], F32)
            nc.vector.reciprocal(inv_sw[:], s_w[:])
            s_x = scal.tile([P, 1], F32)
            nc.vector.tensor_scalar(s_x[:], bc2[:, 1:2], INV_QA, EPS, op0=ALU.mult, op1=ALU.max)
            inv_sx = scal.tile([P, 1], F32)
            nc.vector.reciprocal(inv_sx[:], s_x[:])
            s_xw = scal.tile([P, 1], F32)
            nc.vector.tensor_tensor(out=s_xw[:], in0=s_x[:], in1=s_w[:], op=ALU.mult)

            # ---------------- Quantize helpers (ACT/DVE alternating) ---------
            def quant_chain(src_ap, dst_ap, inv_s, parity):
                mid = midp.tile([P, SB, 512], F32, tag="mid")
                if parity == 0:
                    nc.scalar.activation(
                        mid[:], src_ap, AF.Identity, bias=magic_t[:], scale=inv_s[:]
                    )
                    nc.vector.tensor_scalar(dst_ap, mid[:], -MAGIC, None, op0=ALU.add)
                else:
                    nc.vector.tensor_scalar(
                        mid[:], src_ap, inv_s[:], MAGIC, op0=ALU.mult, op1=ALU.add
                    )
                    nc.scalar.activation(
                        dst_ap, mid[:], AF.Identity, bias=nmagic_t[:], scale=1.0
                    )

            # ---------------- W/X prep + matmul ------------------------------
            qwT = wqp.tile([P, n_kp, JS], BF16, tag="wq", name="qwT")

            def wsrc(i):  # i-th [P,SB,512] w chunk (0..31 across jc0,jc1)
                return wre[i // 2][:, (i % 2) * SB : (i % 2 + 1) * SB, :]

            def xsrc(g, i):  # i-th [P,SB,GT] x chunk of group g
                if g == 0:
                    return xabs[:, i * SB : (i + 1) * SB, :]
                return xre[(g, i // 2)][:, (i % 2) * SB : (i % 2 + 1) * SB, :]

            def wprep_jc(jc):
                for i in range(n_ch):
                    k0 = i * SB
                    quant_chain(
                        wsrc(jc * n_ch + i),
                        qwT[:, k0 : k0 + SB, jc * 512 : (jc + 1) * 512],
                        inv_sw, i % 2,
                    )

            def prep_group(g, qx=None):
                if qx is None:
                    qx = xqp.tile([P, n_kp, GT], BF16, tag="qx", name=f"qx_{g}")
                for i in range(n_ch):
                    k0 = i * SB
                    quant_chain(
                        xsrc(g, i), qx[:, k0 : k0 + SB, :], inv_sx, (i + 1) % 2
                    )
                return qx

            # jc0 + g0 interleaved so both streams advance together
            qx0 = xqp.tile([P, n_kp, GT], BF16, tag="qx", name="qx_0")
            for i in range(n_ch):
                k0 = i * SB
                quant_chain(
                    wsrc(i), qwT[:, k0 : k0 + SB, 0:512], inv_sw, i % 2
                )
                quant_chain(
                    xsrc(0, i), qx0[:, k0 : k0 + SB, :], inv_sx, (i + 1) % 2
                )

            # bias scale + quantized bias (local; needed by first eviction)
            bmax0 = scal.tile([P, 1], F32)
            nc.vector.tensor_reduce(
                bmax0[:], bfull[:], axis=AX.X, op=ALU.max,
                apply_absolute_value=True,
            )
            bmax = scal.tile([P, 1], F32)
            nc.gpsimd.partition_all_reduce(
                bmax[:], bmax0[:], channels=P, reduce_op=bass_isa.ReduceOp.max
            )
            s_b = scal.tile([P, 1], F32)
            nc.vector.tensor_scalar(s_b[:], bmax[:], INV_QA, EPS, op0=ALU.mult, op1=ALU.max)
            inv_sb = scal.tile([P, 1], F32)
            nc.vector.reciprocal(inv_sb[:], s_b[:])
            by = scal.tile([P, n_jt], F32)
            nc.scalar.activation(by[:], bsh[:], AF.Identity, bias=magic_t[:], scale=inv_sb[:])
            bq = scal.tile([P, n_jt], F32)
            nc.vector.tensor_scalar(bq[:], by[:], -MAGIC, s_b[:], op0=ALU.add, op1=ALU.mult)

            # rest of the quantize streams
            wprep_jc(1)
            qx1 = prep_group(1)

            opre = opp.tile([P, n_jt, TS], F16, tag="op", name="opre")
            omax = scal.tile([P, n_g * n_jt], F32)

            def mm_group(g, qx):
                bks = []
                for jt in range(n_jt):
                    bks.append(mmps.tile([P, GT], F32, tag="mm", name=f"mm_{g}_{jt}"))
                for kt in range(n_kp):
                    for jt in range(n_jt):
                        nc.tensor.matmul(
                            bks[jt][:],
                            lhsT=qwT[:, kt, jt * P : (jt + 1) * P],
                            rhs=qx[:, kt, :],
                            start=(kt == 0),
                            stop=(kt == n_kp - 1),
                        )
                return bks

            def evict_group(g, bks):
                for jt in range(n_jt):
                    oc = opre[:, jt, g * GT : (g + 1) * GT]
                    if jt % 2 == 0:
                        nc.scalar.activation(
                            oc, bks[jt][:], AF.Identity,
                            bias=bq[:, jt : jt + 1], scale=s_xw[:],
                        )
                    else:
                        nc.vector.tensor_scalar(
                            oc, bks[jt][:], s_xw[:], bq[:, jt : jt + 1],
                            op0=ALU.mult, op1=ALU.add,
                        )
                    nc.vector.tensor_reduce(
                        omax[:, g * n_jt + jt : g * n_jt + jt + 1], oc,
                        axis=AX.X, op=ALU.max, apply_absolute_value=True,
                    )

            bks0 = mm_group(0, qx0)
            evict_group(0, bks0)
            qx2 = prep_group(2)
            qx3 = prep_group(3)
            bks1 = mm_group(1, qx1)
            evict_group(1, bks1)
            bks2 = mm_group(2, qx2)
            evict_group(2, bks2)
            bks3 = mm_group(3, qx3)
            evict_group(3, bks3)

            # ---------------- Tail: global out absmax -> requantize ---------
            om1 = scal.tile([P, 1], F32)
            nc.vector.tensor_reduce(om1[:], omax[:], axis=AX.X, op=ALU.max)
            omr = scal.tile([P, 1], F32)
            nc.gpsimd.partition_all_reduce(
                omr[:], om1[:], channels=P, reduce_op=bass_isa.ReduceOp.max
            )
            nc.scalar.dma_start(cc2_in[:], omr[:1, :])
            nc.gpsimd.collective_compute(
                "AllReduce", ALU.max, replica_groups=groups,
                ins=[cc2_in[:]], outs=[cc2_out[:]],
            )
            go = scal.tile([P, 1], F32)
            nc.sync.dma_start(go[:1, :], cc2_out[:])
            bco = scal.tile([P, 1], F32)
            nc.gpsimd.partition_broadcast(bco[:], go[:1, :], channels=P)
            s_o = scal.tile([P, 1], F32)
            nc.vector.tensor_scalar(s_o[:], bco[:], INV_QA, EPS, op0=ALU.mult, op1=ALU.max)
            inv_so = scal.tile([P, 1], F32)
            nc.vector.reciprocal(inv_so[:], s_o[:])
            # fp16 magic: out/s_o is in [-127,127], so 1536+v rounds to the
            # integer grid exactly in fp16 (ulp=1 in [1024,2048)).
            M16 = 1536.0
            m16_t = scal.tile([P, 1], F32)
            nc.vector.memset(m16_t[:], M16)
            nbt = scal.tile([P, 1], F32)  # -M16 * s_o for the ACT-second path
            nc.vector.tensor_scalar(nbt[:], s_o[:], -M16, None, op0=ALU.mult)

            # entire requant on DVE: fp16 DVE ops run ~4x faster than ACT
            for jt in range(n_jt):
                src = opre[:, jt, :]
                oy = midp.tile([P, TS], F16, tag="mid")
                nc.vector.tensor_scalar(
                    oy[:], src, inv_so[:], M16, op0=ALU.mult, op1=ALU.add
                )
                res = outst.tile([P, TS], F16, tag="outst")
                nc.vector.tensor_scalar(
                    res[:], oy[:], -M16, s_o[:], op0=ALU.add, op1=ALU.mult
                )
                eng = nc.sync if jt % 2 == 0 else nc.scalar
                eng.dma_start(og_d[jt, :, :], res[:])

    nc.compile()
    return nc


def _tile_pmajor(a2d, n_groups, gw):
    """[K, n_groups*gw] -> [n_groups, 128, K//128, gw] partition-major."""
    K = a2d.shape[0]
    return np.ascontiguousarray(
        a2d.reshape(K // 128, 128, n_groups, gw).transpose(2, 1, 0, 3)
    )


def _run(nc, inputs, n_cores, T, K, J, trace=False):
    from concourse.bass_utils import run_bass_kernel_spmd

    NTG, NJG = 2, 4
    TS, JS = T // NTG, J // NJG
    x = np.ascontiguousarray(inputs["x"], dtype=np.float32)
    w = np.ascontiguousarray(inputs["weight"], dtype=np.float32)
    b = np.ascontiguousarray(inputs["b"], dtype=np.float32)
    xT = np.ascontiguousarray(x.T.astype(np.float16))
    wT = np.ascontiguousarray(w.T)
    in_maps = []
    for c in range(n_cores):
        tg, jgr = divmod(c, NJG)
        xs = xT[:, tg * TS : (tg + 1) * TS]
        ws = wT[:, jgr * JS : (jgr + 1) * JS]
        bs = b[jgr * JS : (jgr + 1) * JS]
        # roll so the exclusive absmax sub-slice is the leading 512 columns
        xrr = np.roll(xs, -jgr * 512, axis=1)
        wrr = np.roll(ws, -tg * 512, axis=1)
        in_maps.append(
            {
                "xg": _tile_pmajor(xrr, TS // 512, 512),
                "wg": _tile_pmajor(wrr, JS // 512, 512),
                "b_full": b,
                "b_shard": np.ascontiguousarray(np.roll(bs, -tg * 512)),
            }
        )
    res = run_bass_kernel_spmd(nc, in_maps, core_ids=list(range(n_cores)), trace=trace)
    out = np.empty((T, J), dtype=np.float32)
    for c in range(n_cores):
        tg, jgr = divmod(c, NJG)
        o = res.results[c]["og"].reshape(JS, TS).astype(np.float32)
        o = np.roll(o, tg * 512, axis=0)
        o = np.roll(o, jgr * 512, axis=1)
        out[tg * TS : (tg + 1) * TS, jgr * JS : (jgr + 1) * JS] = o.T
    return out, res


_NC_CACHE = {}


def kernel(**inputs) -> np.ndarray:
    n_cores, T, K, J = 8, 4096, 4096, 4096
    key = (n_cores, T, K, J)
    if key not in _NC_CACHE:
        _NC_CACHE[key] = build(n_cores, T, K, J)
    out, _ = _run(_NC_CACHE[key], inputs, n_cores, T, K, J)
    return out
